# revision 1
# baseline (speedup 1.0000x reference)
"""MultiHeadAttention (B=4, S=2048, D=2048, H=16) on 8 TRN2 NeuronCores.

Sharding: core c handles batch b = c//2 and head-half = c%2 (8 heads).
Each core computes Q/K/V projections for its 1024 rows, attention for its
8 heads, and a partial output projection; the host sums the two partials
per batch and un-permutes.

Layout trick: torch's `view(B, H, S, dk)` head split (no transpose) means
head h of batch b lives in rows [128h, 128h+128) of the projection output,
with each row holding 16 consecutive seq positions. Working in permuted
query/key coordinates pi = 128*t + u (s = 16*u + t), every attention
operand is an exact 128x128 tile of either the transposed projection
(R^T, for Q/K) or the natural projection (R, for V). Softmax is
permutation-invariant, and the host un-permutes the final output.

All matmuls run in float32r (fp32 with 10-bit mantissa, full PE speed);
host pre-rounds all external matmul operands. Weights are pre-tiled on
the host for contiguous loads; every transfer >256KB is split across DMA
queues. Projection input stages rotate through one double-buffered pool.
"""
import math
import os
from contextlib import ExitStack

import numpy as np

B, S, D, H = 4, 2048, 2048, 16
DK = D // H            # 128
HPC = H // 2           # heads per core = 8
RPC = HPC * DK         # rows per core = 1024
NC_ = 8                # cores
MC = D // 128          # contraction chunks = 16
SCALE = 1.0 / math.sqrt(DK)

_cache = {}
last_results = None


def _round_f32r(x):
    """Round fp32 to the 10-bit-mantissa grid the PE uses for float32r."""
    x = np.ascontiguousarray(x, dtype=np.float32)
    u = x.view(np.uint32)
    lsb = (u >> np.uint32(13)) & np.uint32(1)
    r = (u + np.uint32(0x0FFF) + lsb) & np.uint32(0xFFFFE000)
    return r.view(np.float32)


def _build():
    import concourse.bass as bass
    import concourse.mybir as mybir
    import concourse.tile as tile
    from concourse import bacc

    f32 = mybir.dt.float32
    f32r = mybir.dt.float32r
    AF = mybir.ActivationFunctionType

    nc = bacc.Bacc("TRN2", target_bir_lowering=False, debug=False,
                   num_devices=NC_)

    # ---- external I/O ----
    qts_d = nc.dram_tensor("qts", (MC, 128, RPC), f32r, kind="ExternalInput")
    kts_d = nc.dram_tensor("kts", (MC, 128, RPC), f32r, kind="ExternalInput")
    vts_d = nc.dram_tensor("vts", (MC, 128, RPC), f32r, kind="ExternalInput")
    wqt_d = nc.dram_tensor("wqt", (MC, 128, MC, 128), f32r, kind="ExternalInput")
    wkt_d = nc.dram_tensor("wkt", (MC, 128, MC, 128), f32r, kind="ExternalInput")
    wvt_d = nc.dram_tensor("wvt", (8, 128, MC, 256), f32r, kind="ExternalInput")
    wot_d = nc.dram_tensor("wot", (MC, 128, HPC, 128), f32r, kind="ExternalInput")
    bqs_d = nc.dram_tensor("bqs", (D,), f32, kind="ExternalInput")
    bk_d = nc.dram_tensor("bk", (D,), f32, kind="ExternalInput")
    bvr_d = nc.dram_tensor("bvr", (1, D), f32r, kind="ExternalInput")
    bo_d = nc.dram_tensor("bo", (D,), f32, kind="ExternalInput")
    ones1_d = nc.dram_tensor("ones1", (1, 128), f32r, kind="ExternalInput")
    onescol_d = nc.dram_tensor("onescol", (128, 1), f32r, kind="ExternalInput")
    out_d = nc.dram_tensor("out", (D, S), f32, kind="ExternalOutput")

    with tile.TileContext(nc) as tc, ExitStack() as top:
        rpool = top.enter_context(tc.tile_pool(name="consts", bufs=1))
        dpool = top.enter_context(tc.tile_pool(name="dram", bufs=1, space="DRAM"))

        bq_sb = rpool.tile([128, MC], f32)
        bk_sb = rpool.tile([128, MC], f32)
        bo_sb = rpool.tile([128, MC], f32)
        bv_sb = rpool.tile([1, D], f32r)
        ones1 = rpool.tile([1, 128], f32r)
        onescol = rpool.tile([128, 1], f32r)
        nc.sync.dma_start(bq_sb[:], bqs_d.ap().rearrange("(t p) -> p t", p=128))
        nc.sync.dma_start(bk_sb[:], bk_d.ap().rearrange("(t p) -> p t", p=128))
        nc.sync.dma_start(bo_sb[:], bo_d.ap().rearrange("(t p) -> p t", p=128))
        nc.sync.dma_start(bv_sb[:], bvr_d.ap())
        nc.sync.dma_start(ones1[:], ones1_d.ap())
        nc.sync.dma_start(onescol[:], onescol_d.ap())

        qhat_dram = dpool.tile([MC, 128, RPC], f32r)        # [t][dk][r]
        khat_dram = dpool.tile([HPC, 128, MC, 128], f32r)   # [h][dk][tk][u]
        vhat_dram = dpool.tile([RPC, D], f32r)              # natural R_v

        def load_stage(pool, src_d):
            st = pool.tile([128, MC, RPC], f32r, tag="st")
            for mc in range(MC):
                nc.sync.dma_start(st[:, mc, :], src_d.ap()[mc])
            return st

        def load_w16(pool, src_ap, tag):
            """Load a [128, MC, 128] weight tile in 4 mc-chunks."""
            st = pool.tile([128, MC, 128], f32r, tag=tag)
            for g in range(4):
                nc.sync.dma_start(st[:, 4 * g:4 * g + 4, :],
                                  src_ap[:, 4 * g:4 * g + 4, :])
            return st

        with ExitStack() as stages_es:
            st_pool = stages_es.enter_context(tc.tile_pool(name="stages", bufs=2))
            w_pool = stages_es.enter_context(tc.tile_pool(name="weights", bufs=3))

            # ============= phase Q (first; weights load from t=0) ======
            with ExitStack() as ph:
                qps_pool = ph.enter_context(
                    tc.tile_pool(name="qps", bufs=4, space="PSUM"))
                qout_pool = ph.enter_context(tc.tile_pool(name="qout", bufs=4))
                with nc.named_scope("proj_q"):
                    qt_st = load_stage(st_pool, qts_d)
                    vt_st = load_stage(st_pool, vts_d)   # prefetch for V
                    for ct in range(MC):
                        wq_st = load_w16(w_pool, wqt_d.ap()[ct], "w")
                        for rb in range(2):
                            ps = qps_pool.tile([128, 512], f32, tag="qps")
                            for mc in range(MC):
                                nc.tensor.matmul(
                                    ps[:], wq_st[:, mc, :],
                                    qt_st[:, mc, 512 * rb:512 * rb + 512],
                                    start=(mc == 0), stop=(mc == MC - 1))
                            qo = qout_pool.tile([128, 512], f32r, tag="qo")
                            nc.scalar.activation(qo[:], ps[:], AF.Identity,
                                                 bias=bq_sb[:, ct:ct + 1],
                                                 scale=SCALE)
                            nc.gpsimd.dma_start(
                                qhat_dram[ct, :, 512 * rb:512 * rb + 512],
                                qo[:])

            # ============= phase V (+ prefetch K stage) ================
            with ExitStack() as ph:
                vps_pool = ph.enter_context(
                    tc.tile_pool(name="vps", bufs=4, space="PSUM"))
                vout_pool = ph.enter_context(tc.tile_pool(name="vout", bufs=4))
                with nc.named_scope("proj_v"):
                    kt_st = load_stage(st_pool, kts_d)   # prefetch (rotates)
                    for cb in range(8):           # c blocks of 256
                        wv_st = w_pool.tile([128, MC, 256], f32r, tag="w")
                        for g in range(4):
                            nc.sync.dma_start(
                                wv_st[:, 4 * g:4 * g + 4, :],
                                wvt_d.ap()[cb][:, 4 * g:4 * g + 4, :])
                        for rt in range(8):       # r tiles of 128
                            ps = vps_pool.tile([128, 256], f32, tag="vps")
                            for mc in range(MC):
                                nc.tensor.matmul(
                                    ps[:],
                                    vt_st[:, mc, 128 * rt:128 * rt + 128],
                                    wv_st[:, mc, :], start=(mc == 0),
                                    stop=False)
                            nc.tensor.matmul(
                                ps[:], ones1[:],
                                bv_sb[:, 256 * cb:256 * cb + 256],
                                start=False, stop=True)
                            vo = vout_pool.tile([128, 256], f32r, tag="vo")
                            nc.vector.tensor_copy(vo[:], ps[:])
                            nc.gpsimd.dma_start(
                                vhat_dram[128 * rt:128 * rt + 128,
                                          256 * cb:256 * cb + 256], vo[:])

            # ============= phase K -> khat_dram [h][dk][tk][u] =========
            with ExitStack() as ph:
                kps_pool = ph.enter_context(
                    tc.tile_pool(name="kps", bufs=4, space="PSUM"))
                kout_pool = ph.enter_context(tc.tile_pool(name="kout", bufs=4))
                with nc.named_scope("proj_k"):
                    for ct in range(MC):
                        wk_st = load_w16(w_pool, wkt_d.ap()[ct], "w")
                        for rb in range(2):
                            ps = kps_pool.tile([128, 512], f32, tag="kps")
                            for mc in range(MC):
                                nc.tensor.matmul(
                                    ps[:], wk_st[:, mc, :],
                                    kt_st[:, mc, 512 * rb:512 * rb + 512],
                                    start=(mc == 0), stop=(mc == MC - 1))
                            ko = kout_pool.tile([128, 4, 128], f32r, tag="ko")
                            nc.scalar.activation(ko[:], ps[:], AF.Identity,
                                                 bias=bk_sb[:, ct:ct + 1],
                                                 scale=1.0)
                            dst = khat_dram[4 * rb:4 * rb + 4, :, ct, :] \
                                .rearrange("h p u -> p h u")
                            nc.gpsimd.dma_start(dst, ko[:])

        # ============= attention + output projection ===============
        with ExitStack() as ph:
            q_pool = ph.enter_context(tc.tile_pool(name="qrhs", bufs=4))
            kh_pool = ph.enter_context(tc.tile_pool(name="kh", bufs=3))
            vh_pool = ph.enter_context(tc.tile_pool(name="vh", bufs=3))
            exp_pool = ph.enter_context(tc.tile_pool(name="expp", bufs=6))
            tree_pool = ph.enter_context(tc.tile_pool(name="tree", bufs=2))
            scps_pool = ph.enter_context(
                tc.tile_pool(name="scps", bufs=2, space="PSUM"))
            xps_pool = ph.enter_context(
                tc.tile_pool(name="xps", bufs=2, space="PSUM"))
            sps_pool = ph.enter_context(
                tc.tile_pool(name="sps", bufs=1, space="PSUM"))
            ops_pool = ph.enter_context(
                tc.tile_pool(name="ops", bufs=1, space="PSUM"))
            nrm_pool = ph.enter_context(tc.tile_pool(name="nrm", bufs=2))
            x_pool = ph.enter_context(tc.tile_pool(name="xsb", bufs=4))
            wo_pool = ph.enter_context(tc.tile_pool(name="wo", bufs=3))
            oout_pool = ph.enter_context(tc.tile_pool(name="oout", bufs=3))
            with nc.named_scope("attn"):
                NP_ = MC // 2     # key-tile pairs per head

                def attn_block(j, h, x_j, k_h, v_h, defer_in):
                    q_rhs = q_pool.tile([128, 4, 128], f32r, tag="qr")
                    nc.sync.dma_start(
                        q_rhs[:],
                        qhat_dram[4 * j:4 * j + 4, :, 128 * h:128 * h + 128]
                        .rearrange("t p u -> p t u"))
                    x_ps = xps_pool.tile([128, 512], f32, tag="xps")
                    s_ps = sps_pool.tile([1, 512], f32, tag="sps")
                    acc = tree_pool.tile([128, 2, 512], f32, tag="acc")
                    tsum = tree_pool.tile([128, 512], f32r, tag="tf")
                    exs = [None] * NP_

                    ys = [None] * 4
                    zs = [None] * 2

                    def pv_and_sum(tp):
                        ex = exs[tp]
                        for i in range(2):
                            nc.tensor.matmul(
                                x_ps[:], v_h[:, 2 * tp + i, :],
                                ex[:, i, :], start=(tp == 0 and i == 0),
                                stop=(tp == NP_ - 1 and i == 1))
                        if tp % 2 == 1:
                            a = tp // 2
                            y = tree_pool.tile([128, 2, 512], f32, tag="y")
                            nc.vector.tensor_add(
                                y[:], exs[tp - 1][:].bitcast(f32),
                                ex[:].bitcast(f32))
                            ys[a] = y
                        if tp == 3:
                            z = tree_pool.tile([128, 2, 512], f32, tag="z")
                            nc.vector.tensor_add(z[:], ys[0][:], ys[1][:])
                            zs[0] = z
                        elif tp == NP_ - 1:
                            z = tree_pool.tile([128, 2, 512], f32, tag="z")
                            nc.vector.tensor_add(z[:], ys[2][:], ys[3][:])
                            zs[1] = z
                            nc.vector.tensor_add(acc[:, :, :], zs[0][:],
                                                 zs[1][:])
                            nc.vector.tensor_add(tsum[:], acc[:, 0, :],
                                                 acc[:, 1, :])

                    def finisher():
                        nc.tensor.matmul(s_ps[:], onescol[:], tsum[:],
                                         start=True, stop=True)
                        rec = nrm_pool.tile([1, 512], f32, tag="rec")
                        nc.vector.reciprocal_approx_fast(rec[:], s_ps[:])
                        bcast = nrm_pool.tile([128, 512], f32, tag="bc")
                        nc.gpsimd.partition_broadcast(bcast[:], rec[:])
                        nc.vector.tensor_mul(x_j[:, h, :], x_ps[:], bcast[:])

                    for tp in range(NP_):
                        sc = scps_pool.tile([128, 2, 512], f32, tag="sc")
                        for i in range(2):
                            tk = 2 * tp + i
                            nc.tensor.matmul(
                                sc[:, i, :], k_h[:, tk, :],
                                q_rhs[:, :, :], start=True, stop=True)
                        ex = exp_pool.tile([128, 2, 512], f32r, tag="ex")
                        nc.scalar.activation(ex[:], sc[:], AF.Exp, scale=1.0)
                        exs[tp] = ex
                        if tp == 0:
                            for fn in defer_in:
                                fn()
                        if tp >= 2:
                            pv_and_sum(tp - 2)
                    return [lambda: pv_and_sum(NP_ - 2),
                            lambda: pv_and_sum(NP_ - 1), finisher]

                def emit_otile(j, x_j, ot):
                    wo_st = wo_pool.tile([128, HPC, 128], f32r, tag="wo")
                    for g in range(2):
                        nc.sync.dma_start(
                            wo_st[:, 4 * g:4 * g + 4, :],
                            wot_d.ap()[ot][:, 4 * g:4 * g + 4, :])
                    op = ops_pool.tile([128, 512], f32, tag="op")
                    for h in range(HPC):
                        nc.tensor.matmul(op[:], wo_st[:, h, :],
                                         x_j[:, h, :], start=(h == 0),
                                         stop=(h == HPC - 1))
                    oo = oout_pool.tile([128, 512], f32, tag="oo")
                    nc.scalar.activation(oo[:], op[:], AF.Identity,
                                         bias=bo_sb[:, ot:ot + 1],
                                         scale=1.0)
                    nc.gpsimd.dma_start(
                        out_d.ap()[128 * ot:128 * ot + 128,
                                   512 * j:512 * j + 512], oo[:])

                pending = None    # previous pair's (j0, x0, j1, x1)
                defer = []
                for jp in range(2):           # pairs of query pi-blocks
                    j0, j1 = 2 * jp, 2 * jp + 1
                    x_j0 = x_pool.tile([128, HPC, 512], f32r, tag="xj")
                    x_j1 = x_pool.tile([128, HPC, 512], f32r, tag="xj")
                    for h in range(HPC):
                        k_h = kh_pool.tile([128, MC, 128], f32r, tag="kh")
                        for g in range(4):
                            nc.sync.dma_start(
                                k_h[:, 4 * g:4 * g + 4, :],
                                khat_dram[h][:, 4 * g:4 * g + 4, :])
                        v_h = vh_pool.tile([128, MC, 128], f32r, tag="vh")
                        for g in range(4):
                            nc.sync.dma_start(
                                v_h[:, 4 * g:4 * g + 4, :],
                                vhat_dram[128 * h:128 * h + 128,
                                          512 * g:512 * g + 512])
                        defer = attn_block(j0, h, x_j0, k_h, v_h, defer)
                        if pending is not None:
                            pj0, px0, pj1, px1 = pending
                            emit_otile(pj0, px0, 2 * h)
                            emit_otile(pj1, px1, 2 * h)
                        defer = attn_block(j1, h, x_j1, k_h, v_h, defer)
                        if pending is not None:
                            pj0, px0, pj1, px1 = pending
                            emit_otile(pj0, px0, 2 * h + 1)
                            emit_otile(pj1, px1, 2 * h + 1)
                    pending = (j0, x_j0, j1, x_j1)
                # final pair's output projections
                for fn in defer:
                    fn()
                pj0, px0, pj1, px1 = pending
                for ot in range(MC):
                    emit_otile(pj0, px0, ot)
                    emit_otile(pj1, px1, ot)

    nc.compile()
    return nc


def _prep_shared(Wq, Wk, Wv, Wo, bq, bk, bv, bo):
    wqt = _round_f32r(np.ascontiguousarray(np.asarray(Wq, np.float32).T))
    wkt = _round_f32r(np.ascontiguousarray(np.asarray(Wk, np.float32).T))
    wvt = _round_f32r(np.ascontiguousarray(np.asarray(Wv, np.float32).T))
    wqt_t = np.ascontiguousarray(
        wqt.reshape(MC, 128, MC, 128).transpose(2, 1, 0, 3))
    wkt_t = np.ascontiguousarray(
        wkt.reshape(MC, 128, MC, 128).transpose(2, 1, 0, 3))
    wvt_t = np.ascontiguousarray(
        wvt.reshape(MC, 128, 8, 256).transpose(2, 1, 0, 3))
    woT = np.ascontiguousarray(np.asarray(Wo, np.float32).T)
    bqs = (np.asarray(bq, np.float32) * SCALE).copy()
    bk_np = np.asarray(bk, np.float32).copy()
    bvr = _round_f32r(np.asarray(bv, np.float32).reshape(1, D))
    bo_np = np.asarray(bo, np.float32).copy()
    return wqt_t, wkt_t, wvt_t, woT, bqs, bk_np, bvr, bo_np


def kernel(Q, K, V, Wq, bq, Wk, bk, Wv, bv, Wo, bo, num_heads):
    global last_results
    assert int(num_heads) == H

    from concourse.bass_utils import run_bass_kernel_spmd

    if "nc" not in _cache:
        _cache["nc"] = _build()
    nc = _cache["nc"]

    Q = np.asarray(Q, np.float32)
    K = np.asarray(K, np.float32)
    V = np.asarray(V, np.float32)
    wqt_t, wkt_t, wvt_t, woT, bqs, bk_np, bvr, bo_np = _prep_shared(
        Wq, Wk, Wv, Wo, bq, bk, bv, bo)
    ones1 = np.ones((1, 128), np.float32)
    onescol = np.ones((128, 1), np.float32)

    in_maps = []
    for c in range(NC_):
        b, half = divmod(c, 2)
        r0 = RPC * half
        wot_t = np.ascontiguousarray(
            _round_f32r(woT[r0:r0 + RPC, :])
            .reshape(HPC, 128, MC, 128).transpose(2, 1, 0, 3))
        in_maps.append({
            "qts": _round_f32r(Q[b].T[:, r0:r0 + RPC]).reshape(MC, 128, RPC),
            "kts": _round_f32r(K[b].T[:, r0:r0 + RPC]).reshape(MC, 128, RPC),
            "vts": _round_f32r(V[b].T[:, r0:r0 + RPC]).reshape(MC, 128, RPC),
            "wqt": wqt_t, "wkt": wkt_t, "wvt": wvt_t, "wot": wot_t,
            "bqs": bqs, "bk": bk_np, "bvr": bvr, "bo": bo_np,
            "ones1": ones1, "onescol": onescol,
        })

    res = run_bass_kernel_spmd(nc, in_maps, core_ids=list(range(NC_)))
    last_results = res

    out = np.empty((B, S, D), np.float32)
    for b in range(B):
        oT = res.results[2 * b]["out"] + res.results[2 * b + 1]["out"]
        # oT[o, pi], pi = 128*t + u ; s = 16*u + t
        out[b] = oT.reshape(D, 16, 128).transpose(2, 1, 0).reshape(S, D)
    return out



# revision 2
# speedup vs baseline: 1.0229x; 1.0229x over previous
"""MultiHeadAttention (B=4, S=2048, D=2048, H=16) on 8 TRN2 NeuronCores.

Sharding: core c handles batch b = c//2 and head-half = c%2 (8 heads).
Permuted-coordinate layout trick as v1-v3 (pi = 128*t + u, s = 16*u + t);
host sums two partial output projections per batch and un-permutes.

v4 (over v3): prefetch DMAs (wv weight blocks, wo tiles) are
dependency-gated behind projection evacuations so the critical first
stage half owns the DMA bandwidth (PE starts ~5us in); Q projection's
rb=0 half is interleaved into attention j=0 (head order 4..7,0..3 per j)
using the O-projection PSUM bank, so every attention block has matmul
filler; x is double-buffered across j; output partials are written bf16
and summed in f32 on the host.
"""
import math
from contextlib import ExitStack

import numpy as np

B, S, D, H = 4, 2048, 2048, 16
DK = D // H            # 128
HPC = H // 2           # heads per core = 8
RPC = HPC * DK         # rows per core = 1024
NC_ = 8                # cores
MC = D // 128          # contraction chunks = 16
NP_ = MC // 2          # key-tile pairs per head-block = 8
SCALE = 1.0 / math.sqrt(DK)

_cache = {}
last_results = None


def _bf16():
    import ml_dtypes

    return ml_dtypes.bfloat16


def _build():
    import concourse.bass as bass
    import concourse.mybir as mybir
    import concourse.tile as tile
    from concourse import bacc

    f32 = mybir.dt.float32
    bf16 = mybir.dt.bfloat16
    AF = mybir.ActivationFunctionType

    def gate(waiter_inst, dep_inst, why):
        if waiter_inst is not None and dep_inst is not None:
            tile.add_dep_helper(waiter_inst.ins, dep_inst.ins, sync=True,
                                reason=why)

    nc = bacc.Bacc("TRN2", target_bir_lowering=False, debug=False,
                   num_devices=NC_)

    kts_d = nc.dram_tensor("kts", (MC, 128, RPC), bf16, kind="ExternalInput")
    vts_d = nc.dram_tensor("vts", (MC, 128, RPC), bf16, kind="ExternalInput")
    qts_d = nc.dram_tensor("qts", (MC, 128, RPC), bf16, kind="ExternalInput")
    wkt_d = nc.dram_tensor("wkt", (MC, 128, MC, 128), bf16, kind="ExternalInput")
    wqt_d = nc.dram_tensor("wqt", (MC, 128, MC, 128), bf16, kind="ExternalInput")
    wvt_d = nc.dram_tensor("wvt", (4, 128, MC, 512), bf16, kind="ExternalInput")
    wot_d = nc.dram_tensor("wot", (MC, 128, HPC, 128), bf16, kind="ExternalInput")
    bias_d = nc.dram_tensor("bias", (128, 3 * MC), f32, kind="ExternalInput")
    bvr_d = nc.dram_tensor("bvr", (1, D), f32, kind="ExternalInput")
    out_d = nc.dram_tensor("out", (D, S), bf16, kind="ExternalOutput")

    def half_src(src_d, half, m0, m1):
        return (src_d.ap()[m0:m1, :, 512 * half:512 * half + 512]
                .rearrange("m p r -> p m r"))

    with tile.TileContext(nc) as tc, ExitStack() as top:
        cpool = top.enter_context(tc.tile_pool(name="consts", bufs=1))
        rpool = top.enter_context(tc.tile_pool(name="resident", bufs=1))

        khat = rpool.tile([128, MC, HPC, 128], bf16)   # [d'][tk][h][u]
        vhat = rpool.tile([128, HPC, MC, 128], bf16)   # [u][h][tk][d']
        q_sb = rpool.tile([128, MC, HPC, 128], bf16)   # [d'][tq][h][u]

        bias_sb = cpool.tile([128, 3 * MC], f32)
        bq_sb = bias_sb[:, 0:MC]
        bk_sb = bias_sb[:, MC:2 * MC]
        bo_sb = bias_sb[:, 2 * MC:3 * MC]
        onescol = cpool.tile([128, 1], bf16)
        nc.vector.memset(onescol[:], 1.0)

        with ExitStack() as outer:
            st_pool = outer.enter_context(tc.tile_pool(name="stages", bufs=2))
            w_pool = outer.enter_context(tc.tile_pool(name="wkq", bufs=3))

            # K evacuation instructions, for gating prefetch DMAs
            kevac = {}
            qevac = {}

            with ExitStack() as projes:
                wv_pool = projes.enter_context(tc.tile_pool(name="wv", bufs=2))
                bv_pool = projes.enter_context(tc.tile_pool(name="bvp", bufs=1))
                pps_pool = projes.enter_context(
                    tc.tile_pool(name="pps", bufs=4, space="PSUM"))
                bvr_sb = bv_pool.tile([1, D], f32)
                bvb = bv_pool.tile([128, D], f32)

                # first weight chunk first; first stage half split so the
                # first psum group can start on chunks 0-3
                wk0 = w_pool.tile([128, MC, 128], bf16, tag="w")
                nc.sync.dma_start(wk0[:], wkt_d.ap()[0])
                kt_h = []
                st0 = st_pool.tile([128, MC, 512], bf16, tag="st")
                nc.sync.dma_start(st0[:, 0:4, :], half_src(kts_d, 0, 0, 4))
                nc.sync.dma_start(st0[:, 4:MC, :], half_src(kts_d, 0, 4, MC))
                kt_h.append(st0)
                st1 = st_pool.tile([128, MC, 512], bf16, tag="st")
                kt1_dma = nc.sync.dma_start(st1[:], half_src(kts_d, 1, 0, MC))
                kt_h.append(st1)
                nc.sync.dma_start(bias_sb[:], bias_d.ap())
                nc.sync.dma_start(bvr_sb[:], bvr_d.ap())
                nc.gpsimd.partition_broadcast(bvb[:], bvr_sb[:])

                # ---- K projection -> khat ----
                with nc.named_scope("proj_k"):
                    for rb in range(2):
                        for ct in range(MC):
                            if rb == 0 and ct == 0:
                                wk_st = wk0
                            else:
                                wk_st = w_pool.tile([128, MC, 128], bf16,
                                                    tag="w")
                                nc.sync.dma_start(wk_st[:], wkt_d.ap()[ct])
                            ps = pps_pool.tile([128, 512], f32, tag="ps")
                            for mc in range(MC):
                                nc.tensor.matmul(
                                    ps[:], wk_st[:, mc, :], kt_h[rb][:, mc, :],
                                    start=(mc == 0), stop=(mc == MC - 1))
                            kevac[(rb, ct)] = nc.scalar.activation(
                                khat[:, ct, 4 * rb:4 * rb + 4, :], ps[:],
                                AF.Identity, bias=bk_sb[:, ct:ct + 1],
                                scale=1.0)
                # keep kt half1 off the startup DMA window
                gate(kt1_dma, kevac.get((0, 2)), "kt1 after early K evac")

                # ---- V projection -> vhat (bias via DVE broadcast add) ----
                with nc.named_scope("proj_v"):
                    vt_h = []
                    for half in range(2):
                        st = st_pool.tile([128, MC, 512], bf16, tag="st")
                        nc.sync.dma_start(st[:], half_src(vts_d, half, 0, MC))
                        vt_h.append(st)
                    for half in range(2):
                        for cb in range(4):
                            wv_st = wv_pool.tile([128, MC, 512], bf16,
                                                 tag="wv")
                            wv_dma = nc.sync.dma_start(wv_st[:],
                                                       wvt_d.ap()[cb])
                            gate(wv_dma, kevac[(half, 4 * cb + 2)],
                                 "wv prefetch after K evac")
                            for hl in range(4):
                                h = 4 * half + hl
                                ps = pps_pool.tile([128, 512], f32, tag="ps")
                                for mc in range(MC):
                                    nc.tensor.matmul(
                                        ps[:],
                                        vt_h[half][:, mc,
                                                   128 * hl:128 * hl + 128],
                                        wv_st[:, mc, :], start=(mc == 0),
                                        stop=(mc == MC - 1))
                                nc.vector.tensor_add(
                                    vhat[:, h, 4 * cb:4 * cb + 4, :], ps[:],
                                    bvb[:, 512 * cb:512 * cb + 512])

                # ---- Q projection rb=1 only (rb=0 interleaves into attn) ----
                with nc.named_scope("proj_q"):
                    # half 1 first: it feeds the main (rb=1) phase, and the
                    # first-loaded tile gets the earlier-released stage slot
                    qt_h = [None, None]
                    for half in (1, 0):
                        st = st_pool.tile([128, MC, 512], bf16, tag="st")
                        nc.sync.dma_start(st[:], half_src(qts_d, half, 0, MC))
                        qt_h[half] = st
                    for ct in range(MC):
                        wq_st = w_pool.tile([128, MC, 128], bf16, tag="w")
                        nc.sync.dma_start(wq_st[:], wqt_d.ap()[ct])
                        ps = pps_pool.tile([128, 512], f32, tag="ps")
                        for mc in range(MC):
                            nc.tensor.matmul(ps[:], wq_st[:, mc, :],
                                             qt_h[1][:, mc, :],
                                             start=(mc == 0), stop=(mc == MC - 1))
                        qevac[(1, ct)] = nc.scalar.activation(
                            q_sb[:, ct, 4:8, :], ps[:], AF.Identity,
                            bias=bq_sb[:, ct:ct + 1], scale=SCALE)

            # ---- attention + output projection (+ Q proj rb=0) ----
            with ExitStack() as aouter:
                x_pool = aouter.enter_context(tc.tile_pool(name="xsb", bufs=1))
                wo_pool = aouter.enter_context(tc.tile_pool(name="wo", bufs=4))
                oout_pool = aouter.enter_context(
                    tc.tile_pool(name="oout", bufs=3))
                x_sb = x_pool.tile([128, HPC, 2, 512], bf16)  # [d'][h][j%2][q]

                def emit_otile(ops_pool, j, ot, gate_dep=None):
                    wo_st = wo_pool.tile([128, HPC, 128], bf16, tag="wo")
                    wo_dma = nc.sync.dma_start(wo_st[:], wot_d.ap()[ot])
                    gate(wo_dma, gate_dep, "wo prefetch gate")
                    op = ops_pool.tile([128, 512], f32, tag="op")
                    for h in range(HPC):
                        nc.tensor.matmul(op[:], wo_st[:, h, :],
                                         x_sb[:, h, j % 2, :],
                                         start=(h == 0), stop=(h == HPC - 1))
                    oo = oout_pool.tile([128, 512], bf16, tag="oo")
                    nc.vector.tensor_scalar(oo[:], op[:], bo_sb[:, ot:ot + 1],
                                            None, mybir.AluOpType.add)
                    nc.sync.dma_start(
                        out_d.ap()[128 * ot:128 * ot + 128,
                                   512 * j:512 * j + 512], oo[:])

                with ExitStack() as ph, nc.named_scope("attn"):
                    exp_pool = ph.enter_context(
                        tc.tile_pool(name="expp", bufs=6))
                    tree_pool = ph.enter_context(
                        tc.tile_pool(name="tree", bufs=2))
                    scps_pool = ph.enter_context(
                        tc.tile_pool(name="scps", bufs=2, space="PSUM"))
                    xps_pool = ph.enter_context(
                        tc.tile_pool(name="xps", bufs=2, space="PSUM"))
                    sps_pool = ph.enter_context(
                        tc.tile_pool(name="sps", bufs=1, space="PSUM"))
                    ops_pool = ph.enter_context(
                        tc.tile_pool(name="ops", bufs=1, space="PSUM"))
                    nrm_pool = ph.enter_context(tc.tile_pool(name="nrm",
                                                             bufs=2))

                    def attn_block(j, h, defer_in):
                        x_ps = xps_pool.tile([128, 512], f32, tag="xps")
                        s_ps = sps_pool.tile([1, 512], f32, tag="sps")
                        acc = tree_pool.tile([128, 2, 512], bf16, tag="acc")
                        tsum = tree_pool.tile([128, 512], bf16, tag="tf")
                        exs = [None] * NP_
                        ys = [None] * 4
                        zs = [None] * 2
                        q_rhs = q_sb[:, 4 * j:4 * j + 4, h, :]

                        def pv_and_sum(tp):
                            ex = exs[tp]
                            for i in range(2):
                                nc.tensor.matmul(
                                    x_ps[:], vhat[:, h, 2 * tp + i, :],
                                    ex[:, i, :], start=(tp == 0 and i == 0),
                                    stop=(tp == NP_ - 1 and i == 1))
                            if tp % 2 == 1:
                                a = tp // 2
                                y = tree_pool.tile([128, 2, 512], bf16,
                                                   tag="y")
                                nc.vector.tensor_add(y[:], exs[tp - 1][:],
                                                     ex[:])
                                ys[a] = y
                            if tp == 3:
                                z = tree_pool.tile([128, 2, 512], bf16,
                                                   tag="z")
                                nc.vector.tensor_add(z[:], ys[0][:], ys[1][:])
                                zs[0] = z
                            elif tp == NP_ - 1:
                                z = tree_pool.tile([128, 2, 512], bf16,
                                                   tag="z")
                                nc.vector.tensor_add(z[:], ys[2][:], ys[3][:])
                                zs[1] = z
                                nc.vector.tensor_add(acc[:, :, :], zs[0][:],
                                                     zs[1][:])
                                nc.vector.tensor_add(tsum[:], acc[:, 0, :],
                                                     acc[:, 1, :])

                        def finisher():
                            nc.tensor.matmul(s_ps[:], onescol[:], tsum[:],
                                             start=True, stop=True)
                            rec = nrm_pool.tile([1, 512], f32, tag="rec")
                            nc.vector.reciprocal_approx_fast(rec[:], s_ps[:])
                            bcast = nrm_pool.tile([128, 512], f32, tag="bc")
                            nc.gpsimd.partition_broadcast(bcast[:], rec[:])
                            nc.vector.tensor_mul(x_sb[:, h, j % 2, :],
                                                 x_ps[:], bcast[:])

                        for tp in range(NP_):
                            sc = scps_pool.tile([128, 2, 512], f32, tag="sc")
                            for i in range(2):
                                tk = 2 * tp + i
                                nc.tensor.matmul(sc[:, i, :],
                                                 khat[:, tk, h, :],
                                                 q_rhs, start=True, stop=True)
                            ex = exp_pool.tile([128, 2, 512], bf16, tag="ex")
                            nc.scalar.activation(ex[:], sc[:], AF.Exp,
                                                 scale=1.0)
                            exs[tp] = ex
                            if tp == 0:
                                for fn in defer_in:
                                    fn()
                            if tp >= 2:
                                pv_and_sum(tp - 2)
                        return [lambda: pv_and_sum(NP_ - 2),
                                lambda: pv_and_sum(NP_ - 1), finisher]

                    HORD = [4, 5, 6, 7, 0, 1, 2, 3]
                    defer = []
                    for j in range(4):
                        for hp, h in enumerate(HORD):
                            defer = attn_block(j, h, defer)
                            if j == 0:
                                # Q projection rb=0, two ct-groups per block
                                for ct in (2 * hp, 2 * hp + 1):
                                    wq_st = w_pool.tile([128, MC, 128], bf16,
                                                        tag="w")
                                    nc.sync.dma_start(wq_st[:],
                                                      wqt_d.ap()[ct])
                                    ps = ops_pool.tile([128, 512], f32,
                                                       tag="op")
                                    for mc in range(MC):
                                        nc.tensor.matmul(
                                            ps[:], wq_st[:, mc, :],
                                            qt_h[0][:, mc, :],
                                            start=(mc == 0),
                                            stop=(mc == MC - 1))
                                    qevac[(0, ct)] = nc.scalar.activation(
                                        q_sb[:, ct, 0:4, :], ps[:],
                                        AF.Identity,
                                        bias=bq_sb[:, ct:ct + 1], scale=SCALE)
                            else:
                                gd = qevac[(0, 15)] if j == 1 else None
                                emit_otile(ops_pool, j - 1, 2 * hp, gd)
                                emit_otile(ops_pool, j - 1, 2 * hp + 1, gd)
                    for fn in defer:
                        fn()

                with ExitStack() as tl:
                    ops2_pool = tl.enter_context(
                        tc.tile_pool(name="ops2", bufs=2, space="PSUM"))
                    for ot in range(MC):
                        emit_otile(ops2_pool, 3, ot)

    nc.compile()
    return nc


def _prep_shared(Wq, Wk, Wv, Wo, bq, bk, bv, bo):
    bf16 = _bf16()
    Wq = np.asarray(Wq, np.float32)
    Wk = np.asarray(Wk, np.float32)
    Wv = np.asarray(Wv, np.float32)
    Wo = np.asarray(Wo, np.float32)
    wqt = np.ascontiguousarray(
        Wq.reshape(MC, 128, MC, 128).transpose(0, 3, 2, 1)).astype(bf16)
    wkt = np.ascontiguousarray(
        Wk.reshape(MC, 128, MC, 128).transpose(0, 3, 2, 1)).astype(bf16)
    wvt = np.ascontiguousarray(
        Wv.reshape(4, 512, MC, 128).transpose(0, 3, 2, 1)).astype(bf16)
    wo4 = Wo.reshape(MC, 128, MC, 128)
    wot = [np.ascontiguousarray(
        wo4[:, :, 8 * half:8 * half + 8, :].transpose(0, 3, 2, 1)).astype(bf16)
        for half in range(2)]
    bias = np.empty((128, 3 * MC), np.float32)
    bias[:, 0:MC] = (np.asarray(bq, np.float32) * SCALE).reshape(MC, 128).T
    bias[:, MC:2 * MC] = np.asarray(bk, np.float32).reshape(MC, 128).T
    bias[:, 2 * MC:3 * MC] = np.asarray(bo, np.float32).reshape(MC, 128).T
    bvr = np.asarray(bv, np.float32).reshape(1, D).copy()
    return wqt, wkt, wvt, wot, bias, bvr


def kernel(Q, K, V, Wq, bq, Wk, bk, Wv, bv, Wo, bo, num_heads):
    global last_results
    assert int(num_heads) == H

    from concourse.bass_utils import run_bass_kernel_spmd

    if "nc" not in _cache:
        _cache["nc"] = _build()
    nc = _cache["nc"]

    bf16 = _bf16()
    Q = np.asarray(Q, np.float32)
    K = np.asarray(K, np.float32)
    V = np.asarray(V, np.float32)
    wqt, wkt, wvt, wot, bias, bvr = _prep_shared(
        Wq, Wk, Wv, Wo, bq, bk, bv, bo)

    in_maps = []
    for c in range(NC_):
        b, half = divmod(c, 2)
        r0 = RPC * half
        in_maps.append({
            "qts": np.ascontiguousarray(Q[b].T[:, r0:r0 + RPC]).astype(bf16)
            .reshape(MC, 128, RPC),
            "kts": np.ascontiguousarray(K[b].T[:, r0:r0 + RPC]).astype(bf16)
            .reshape(MC, 128, RPC),
            "vts": np.ascontiguousarray(V[b].T[:, r0:r0 + RPC]).astype(bf16)
            .reshape(MC, 128, RPC),
            "wqt": wqt, "wkt": wkt, "wvt": wvt, "wot": wot[half],
            "bias": bias, "bvr": bvr,
        })

    res = run_bass_kernel_spmd(nc, in_maps, core_ids=list(range(NC_)))
    last_results = res

    out = np.empty((B, S, D), np.float32)
    for b in range(B):
        oT = (np.asarray(res.results[2 * b]["out"], np.float32)
              + np.asarray(res.results[2 * b + 1]["out"], np.float32))
        # oT[o, pi], pi = 128*t + u ; s = 16*u + t
        out[b] = oT.reshape(D, 16, 128).transpose(2, 1, 0).reshape(S, D)
    return out
